# revision 25
# baseline (speedup 1.0000x reference)
"""MoE decoder kernel for Trainium2 (8 NeuronCores, expert-parallel).

Strategy
--------
Host (numpy): gate (sigmoid + top-8 + weight normalization), token->expert
dispatch, weight repacking in PE-friendly layout, final scatter-add
combine + LayerNorm.

Device (Bass/Tile, SPMD over 8 cores): 8 experts per core.  For each
expert the 4-layer MLP runs with *feature-major* activations
(act^T: [feat, tokens]) so that every matmul uses the natural-layout
weight tile [K=128, M=128] as the stationary operand and the activation
tile [K=128, T] as the moving operand -- no transposes anywhere.
Weights stream HBM->SBUF exactly once per expert as large contiguous
"mega chunk" DMAs, double-buffered through ring pools.

W2 (the 2048x2048 layer) and the top 5/8 of W1's contraction dim are
shipped as fp8-e3m4 (4 mantissa bits) with global power-of-2 scales --
W2's scale is undone inside the L2 gelu's `scale` operand, W1's by
pre-scaling the matching x rows on the host (exact in bf16).  The PE
accepts the mixed fp8-stationary x bf16-moving matmul natively.
Outputs store as bf16.  HBM traffic per core: 112MB -> ~70MB, max rel
err ~1.8e-2 (gate: 2e-2; the host sim of this exact quantization has
matched the device result to 4+ digits on every run).

Output stores issue from the GpSimd SWDGE so the sync-ring load stream
is never blocked behind a compute-dependent store.  The first slot's
x/w1 loads are split into k-chunks to shorten the startup ramp, and the
last (smallest) slot is split in half so the final L2/L3/L4 chain
overlaps with remaining compute, shortening the tail.

Per (token,expert) pair only selected pairs are computed (capacity =
per-slot max over cores, rounded to 8), so compute ~= the sparse top-8
workload.
"""

import math

import numpy as np
import ml_dtypes

# problem constants (hardcoded; kernel.py must be self-contained)
B, S, D = 2, 512, 1024
H, BN, O = 2048, 256, 768
E, TOPK = 64, 8
N = B * S
NCORES = 8
EPC = E // NCORES  # experts per core

BF16 = ml_dtypes.bfloat16
E3M4 = ml_dtypes.float8_e3m4

LAST_EXEC_NS = None  # test harness reads this after a traced run
LAST_RES = None  # full BassKernelResults for offline trace analysis


# ---------------------------------------------------------------------------
# host-side routing
# ---------------------------------------------------------------------------

def _route(x, gate_w, gate_bias):
    """Replicates the reference gate in float64: returns top_idx [N,8],
    combine weights wc [N,8] (float32)."""
    xf = x.reshape(N, D).astype(np.float64)
    logits = xf @ gate_w.astype(np.float64).T
    scores = 1.0 / (1.0 + np.exp(-logits))
    choice = scores + gate_bias.astype(np.float64)[None, :]
    # top-8, descending, stable (matches jax.lax.top_k tie behavior)
    top_idx = np.argsort(-choice, axis=1, kind="stable")[:, :TOPK]
    top_scores = np.take_along_axis(choice, top_idx, axis=1)
    wc = top_scores / (top_scores.sum(-1, keepdims=True) + 1e-6)
    return top_idx.astype(np.int64), wc.astype(np.float32)


def _assign_experts(counts):
    """Greedy balance: experts -> cores (EPC slots each), sorted desc within
    a core.  Returns assign[core][slot] = expert id."""
    order = np.argsort(-counts, kind="stable")
    loads = [0] * NCORES
    nslot = [0] * NCORES
    assign = [[] for _ in range(NCORES)]
    for e in order:
        # least-loaded core with a free slot
        c = min(
            (c for c in range(NCORES) if nslot[c] < EPC),
            key=lambda c: (loads[c], c),
        )
        assign[c].append(int(e))
        loads[c] += int(counts[e])
        nslot[c] += 1
    return assign  # each list already desc by count (greedy order)


def _make_slots(caps):
    """Slot schedule: (expert_row ri, cap, token_start).  Experts 0-6 are one
    slot each; the last (smallest) expert is split in two so its final
    L2/L3/L4 chain overlaps the first half's compute."""
    slots = []
    for r in range(EPC - 1):
        slots.append((r, int(caps[r]), 0))
    C7 = int(caps[EPC - 1])
    a = ((C7 // 2 + 7) // 8) * 8
    if C7 - a >= 8:
        slots.append((EPC - 1, a, 0))
        slots.append((EPC - 1, C7 - a, a))
    else:
        slots.append((EPC - 1, C7, 0))
    return slots


# ---------------------------------------------------------------------------
# device program
# ---------------------------------------------------------------------------

def _build_program(slots, inv_s2):
    import concourse.bass as bass
    import concourse.tile as tile
    from concourse import mybir

    DT = mybir.dt.bfloat16
    F8 = mybir.dt.float8e3
    F32 = mybir.dt.float32
    SC = sum(C for _, C, _ in slots)
    xoffs = np.concatenate([[0], np.cumsum([8 * C for _, C, _ in slots])])
    ooffs = np.concatenate([[0], np.cumsum([6 * C for _, C, _ in slots])])

    nc = bass.Bass(trn_type="TRN2")
    w1s = nc.dram_tensor("w1s", [EPC, 2, 128, 3072], DT, kind="ExternalInput")
    w1f = nc.dram_tensor("w1f", [EPC, 2, 128, 5120], F8, kind="ExternalInput")
    w2s = nc.dram_tensor("w2s", [EPC, 4, 128, 8192], F8, kind="ExternalInput")
    w3s = nc.dram_tensor("w3s", [EPC, 4, 128, 1024], DT, kind="ExternalInput")
    w4s = nc.dram_tensor("w4s", [EPC, 128, 1536], DT, kind="ExternalInput")
    xt = nc.dram_tensor("xt", [128, 8 * SC], DT, kind="ExternalInput")
    bias = nc.dram_tensor("bias", [128, EPC * 40], F32, kind="ExternalInput")
    out = nc.dram_tensor("out", [128, 6 * SC], DT, kind="ExternalOutput")

    GELU = mybir.ActivationFunctionType.Gelu

    with tile.TileContext(nc) as tc:
        with (
            tc.tile_pool(name="wt", bufs=10) as wpool,
            tc.tile_pool(name="wt8", bufs=8) as w8pool,
            tc.tile_pool(name="w1f8", bufs=4) as w1fpool,
            tc.tile_pool(name="xtp", bufs=3) as xpool,
            tc.tile_pool(name="h1p", bufs=32) as h1pool,
            tc.tile_pool(name="h2p", bufs=32) as h2pool,
            tc.tile_pool(name="h3p", bufs=4) as h3pool,
            tc.tile_pool(name="outp", bufs=3) as opool,
            tc.tile_pool(name="ps", bufs=8, space="PSUM") as pspool,
            tc.tile_pool(name="one", bufs=1) as single,
        ):
            bias_sb = single.tile([128, EPC * 40], F32)
            # bias rides the GpSimd ring so the sync ring starts on x/w1
            # immediately
            nc.gpsimd.dma_start(out=bias_sb, in_=bias[:, :])
            # Observer ops: ACT and DVE each touch the bias tile once so the
            # bias-DMA tick is already observed by those engines -- keeps every
            # later activation/tensor_scalar at <=1 sync wait (the legacy
            # walrus codegen rejects instructions with 2+ waits).
            obs_a = single.tile([128, 1], F32)
            nc.scalar.copy(out=obs_a, in_=bias_sb[:, 0:1])
            obs_v = single.tile([128, 1], F32)
            nc.vector.tensor_copy(out=obs_v, in_=bias_sb[:, 0:1])

            psn = [0]

            def psum_alloc(n, C):
                outs = []
                for _ in range(n):
                    psn[0] += 1
                    outs.append(
                        pspool.tile([128, C], F32, tag="ps", name=f"ps{psn[0]}")
                    )
                return outs

            for si, (ri, C, _) in enumerate(slots):
                xoff = int(xoffs[si])
                ooff = int(ooffs[si])
                bcol = ri * 40

                # gathered tokens, transposed: 8 k-tiles of [128, C]
                xtile = xpool.tile([128, 8 * C], DT, tag="xt")
                if si == 0:
                    # startup ramp: land k-tile 0 first so the first matmul
                    # can begin after ~0.3MB instead of ~1.4MB
                    nc.sync.dma_start(out=xtile[:, :C], in_=xt[:, xoff: xoff + C])
                    nc.sync.dma_start(
                        out=xtile[:, C:], in_=xt[:, xoff + C: xoff + 8 * C]
                    )
                else:
                    nc.sync.dma_start(out=xtile, in_=xt[:, xoff: xoff + 8 * C])
                xts = [xtile[:, k * C:(k + 1) * C] for k in range(8)]

                # ---- L1: h1^T[H, C] = gelu(W1^T x + b1), K=D (8 tiles);
                # k0-2 bf16, k3-7 e3m4 (x rows carry the exact 1/s1) ----
                h1 = []
                for g in range(2):  # m-groups of 8 feature tiles
                    psums = psum_alloc(8, C)
                    lo = wpool.tile([128, 3072], DT, tag="wt")
                    if si == 0 and g == 0:
                        for c in range(3):
                            nc.sync.dma_start(
                                out=lo[:, c * 1024:(c + 1) * 1024],
                                in_=w1s[ri, g, :, c * 1024:(c + 1) * 1024],
                            )
                    else:
                        nc.sync.dma_start(out=lo, in_=w1s[ri, g])
                    for c in range(3):
                        k = c
                        for m in range(8):
                            nc.tensor.matmul(
                                psums[m],
                                lo[:, c * 1024 + m * 128: c * 1024 + (m + 1) * 128],
                                xts[k],
                                start=(k == 0),
                                stop=False,
                            )
                    hi = w1fpool.tile([128, 5120], F8, tag="w1f8")
                    nc.sync.dma_start(out=hi, in_=w1f[ri, g])
                    for c in range(5):
                        k = 3 + c
                        for m in range(8):
                            nc.tensor.matmul(
                                psums[m],
                                hi[:, c * 1024 + m * 128: c * 1024 + (m + 1) * 128],
                                xts[k],
                                start=False,
                                stop=(k == 7),
                            )
                    for m in range(8):
                        hh = h1pool.tile([128, C], DT, tag="h1")
                        nc.scalar.activation(
                            out=hh, in_=psums[m], func=GELU,
                            bias=bias_sb[:, bcol + g * 8 + m: bcol + g * 8 + m + 1],
                        )
                        h1.append(hh)

                # ---- L2: h2^T[H, C] = gelu(W2^T h1 + b2), K=H (16 tiles),
                # W2 in fp8-e3m4 scaled by s2 (undone via activation scale) --
                h2 = []
                for g in range(2):
                    psums = psum_alloc(8, C)
                    for mega in range(2):  # 2 megas x 8 k-chunks
                        wt = w8pool.tile([128, 8192], F8, tag="wt8")
                        nc.sync.dma_start(out=wt, in_=w2s[ri, g * 2 + mega])
                        for c in range(8):
                            k = mega * 8 + c
                            for m in range(8):
                                nc.tensor.matmul(
                                    psums[m],
                                    wt[:, c * 1024 + m * 128: c * 1024 + (m + 1) * 128],
                                    h1[k],
                                    start=(k == 0),
                                    stop=(k == 15),
                                )
                    for m in range(8):
                        hh = h2pool.tile([128, C], DT, tag="h2")
                        nc.scalar.activation(
                            out=hh, in_=psums[m], func=GELU, scale=float(inv_s2),
                            bias=bias_sb[:, bcol + 16 + g * 8 + m: bcol + 16 + g * 8 + m + 1],
                        )
                        h2.append(hh)

                # ---- L3: h3^T[BN, C] = W3^T h2 + b3, K=H (16 tiles) ----
                psums3 = psum_alloc(2, C)
                for mega in range(4):
                    wt = wpool.tile([128, 1024], DT, tag="wt")
                    nc.sync.dma_start(out=wt, in_=w3s[ri, mega])
                    for c in range(4):
                        k = mega * 4 + c
                        for m in range(2):
                            nc.tensor.matmul(
                                psums3[m],
                                wt[:, c * 256 + m * 128: c * 256 + (m + 1) * 128],
                                h2[k],
                                start=(k == 0),
                                stop=(k == 15),
                            )
                h3 = []
                for m in range(2):
                    hh = h3pool.tile([128, C], DT, tag="h3")
                    nc.vector.tensor_scalar_add(
                        hh, psums3[m], bias_sb[:, bcol + 32 + m: bcol + 32 + m + 1]
                    )
                    h3.append(hh)

                # ---- L4: out^T[O, C] = W4^T h3 + b4, K=BN (2 tiles) ----
                psums4 = psum_alloc(6, C)
                wt = wpool.tile([128, 1536], DT, tag="wt")
                nc.sync.dma_start(out=wt, in_=w4s[ri])
                for c in range(2):
                    for m in range(6):
                        nc.tensor.matmul(
                            psums4[m],
                            wt[:, c * 768 + m * 128: c * 768 + (m + 1) * 128],
                            h3[c],
                            start=(c == 0),
                            stop=(c == 1),
                        )
                ot = opool.tile([128, 6 * C], DT, tag="out")
                for m in range(6):
                    nc.vector.tensor_scalar_add(
                        ot[:, m * C:(m + 1) * C], psums4[m],
                        bias_sb[:, bcol + 34 + m: bcol + 34 + m + 1],
                    )
                # store via GpSimd (SWDGE): a store's sequencer-side wait on
                # the DVE adds must not stall the sync-engine FIFO, which
                # carries every weight load for the NEXT experts.
                nc.gpsimd.dma_start(
                    out=out[:, ooff: ooff + 6 * C], in_=ot
                )

    _legalize_waits(nc, mybir)
    return nc


def _legalize_waits(nc, mybir):
    """The legacy walrus codegen (bass2jax path) rejects instructions carrying
    more than one sync wait.  Split every multi-wait instruction: hoist all
    but the last wait onto same-engine InstNoOp carriers inserted just before
    it (engine program order preserves the gating semantics)."""
    n = 0
    for bb in nc.main_func.blocks:
        insts = bb.instructions
        i = 0
        while i < len(insts):
            ins = insts[i]
            si = ins.sync_info
            if si is not None and si.on_wait and len(si.on_wait) > 1:
                extra = list(si.on_wait[:-1])
                keep = [si.on_wait[-1]]
                for w in extra:
                    noop = mybir.InstNoOp(
                        name=f"NOPW-{n}", engine=ins.engine, ins=[], outs=[],
                        sync_info=mybir.SyncInfo(on_wait=[w], on_update=[]),
                    )
                    n += 1
                    insts.insert(i, noop)
                    i += 1
                ins.sync_info = mybir.SyncInfo(
                    on_wait=keep, on_update=list(si.on_update or [])
                )
            i += 1


# ---------------------------------------------------------------------------
# host-side packing
# ---------------------------------------------------------------------------

def _pack_core(w1, b1, w2, b2, w3, b3, w4, b4, experts, s1, s2):
    """Pack one core's 8 experts into the DRAM layouts the program expects."""
    idx = np.asarray(experts)
    # W1 [e,1024,2048]: chunks (g,k) of [128,1024]; k0-2 -> bf16 mega,
    # k3-7 -> e3m4 mega scaled by s1 (undone via the 1/s1-scaled x rows)
    a = w1[idx].reshape(EPC, 8, 128, 2, 1024)
    a = a.transpose(0, 3, 1, 2, 4)  # [EPC, g, k, 128, 1024]
    lo = a[:, :, :3].transpose(0, 1, 3, 2, 4).reshape(EPC, 2, 128, 3072)
    hi = a[:, :, 3:].transpose(0, 1, 3, 2, 4).reshape(EPC, 2, 128, 5120)
    w1p = np.ascontiguousarray(lo).astype(BF16)
    w1fp = np.clip(np.ascontiguousarray(hi) * s1, -15.5, 15.5).astype(E3M4)

    # W2 [e,2048,2048] -> fp8-e3m4 [e,4,128,8192]: 8 k-chunks of [128,1024]
    # per mega, scaled by s2
    a = w2[idx].reshape(EPC, 16, 128, 2, 1024)
    a = a.transpose(0, 3, 1, 2, 4).reshape(EPC, 32, 128, 1024)
    a = a.reshape(EPC, 4, 8, 128, 1024).transpose(0, 1, 3, 2, 4)
    w2p = np.clip(
        np.ascontiguousarray(a).reshape(EPC, 4, 128, 8192) * s2, -15.5, 15.5
    ).astype(E3M4)

    a = w3[idx].reshape(EPC, 16, 128, 256)  # k-chunks of [128,256]
    w3p = np.ascontiguousarray(
        a.reshape(EPC, 4, 4, 128, 256).transpose(0, 1, 3, 2, 4)
    ).reshape(EPC, 4, 128, 1024).astype(BF16)

    a = w4[idx].reshape(EPC, 2, 128, 768)
    w4p = np.ascontiguousarray(a.transpose(0, 2, 1, 3)).reshape(
        EPC, 128, 1536
    ).astype(BF16)

    # biases: per expert 40 cols of [128]: L1 m0-15 | L2 m0-15 | L3 m0-1 | L4 m0-5
    bb = np.concatenate(
        [
            b1[idx].reshape(EPC, 16, 128),
            b2[idx].reshape(EPC, 16, 128),
            b3[idx].reshape(EPC, 2, 128),
            b4[idx].reshape(EPC, 6, 128),
        ],
        axis=1,
    )  # [EPC, 40, 128]
    biasp = np.ascontiguousarray(
        bb.reshape(EPC * 40, 128).T
    ).astype(np.float32)  # [128, EPC*40]
    return w1p, w1fp, w2p, w3p, w4p, biasp


def kernel(x, gate_w, gate_bias, w1, b1, w2, b2, w3, b3, w4, b4, ln_w, ln_b):
    global LAST_EXEC_NS
    x = np.asarray(x, np.float32)
    xf = x.reshape(N, D)

    top_idx, wc = _route(x, np.asarray(gate_w, np.float32),
                         np.asarray(gate_bias, np.float32))

    # token lists per expert
    counts = np.bincount(top_idx.ravel(), minlength=E)
    tok_of = [[] for _ in range(E)]
    w_of = [[] for _ in range(E)]
    flat_tok = np.repeat(np.arange(N), TOPK)
    flat_exp = top_idx.ravel()
    flat_w = wc.ravel()
    order = np.argsort(flat_exp, kind="stable")
    for t, e, w in zip(flat_tok[order], flat_exp[order], flat_w[order]):
        tok_of[e].append(int(t))
        w_of[e].append(float(w))

    assign = _assign_experts(counts)

    # per-slot capacities (shared across cores; slots sorted desc by count)
    caps = np.zeros(EPC, int)
    for c in range(NCORES):
        for r, e in enumerate(assign[c]):
            caps[r] = max(caps[r], counts[e])
    caps = ((caps + 7) // 8) * 8
    slots = _make_slots(caps)
    SC = sum(C for _, C, _ in slots)
    xoffs = np.concatenate([[0], np.cumsum([8 * C for _, C, _ in slots])])
    ooffs = np.concatenate([[0], np.cumsum([6 * C for _, C, _ in slots])])

    w1a = np.asarray(w1, np.float32); b1a = np.asarray(b1, np.float32)
    w2a = np.asarray(w2, np.float32); b2a = np.asarray(b2, np.float32)
    w3a = np.asarray(w3, np.float32); b3a = np.asarray(b3, np.float32)
    w4a = np.asarray(w4, np.float32); b4a = np.asarray(b4, np.float32)

    # global power-of-2 scales for the fp8-e3m4 weights (max normal 15.5)
    s1 = 2.0 ** math.floor(math.log2(15.0 / float(np.abs(w1a).max())))
    s2 = 2.0 ** math.floor(math.log2(15.0 / float(np.abs(w2a).max())))

    nc = _build_program(slots, 1.0 / s2)

    xt_bf = xf.T.astype(BF16)  # [D, N]
    # x rows 384-1023 (k-tiles 3-7) carry the exact 1/s1 to undo the fp8
    # part of W1's scale; power-of-2 so the bf16 cast stays lossless
    xt_bf[384:] = (xt_bf[384:].astype(np.float32) * (1.0 / s1)).astype(BF16)
    in_maps = []
    for c in range(NCORES):
        w1p, w1fp, w2p, w3p, w4p, biasp = _pack_core(
            w1a, b1a, w2a, b2a, w3a, b3a, w4a, b4a, assign[c], s1, s2
        )
        # xt layout: per slot, 8 k-tiles [128, C] side by side
        xtc = np.zeros((128, 8 * int(SC)), BF16)
        for si, (ri, C, tstart) in enumerate(slots):
            e = assign[c][ri]
            ids = tok_of[e][tstart:tstart + C]
            if not ids:
                continue
            blk = xt_bf[:, ids]  # [D, n]
            for k in range(8):
                base = int(xoffs[si]) + k * C
                xtc[:, base: base + len(ids)] = blk[k * 128:(k + 1) * 128]
        in_maps.append(
            {"w1s": w1p, "w1f": w1fp, "w2s": w2p, "w3s": w3p, "w4s": w4p,
             "xt": xtc, "bias": biasp}
        )

    from concourse.bass_utils import run_bass_kernel_spmd

    res = run_bass_kernel_spmd(nc, in_maps, core_ids=list(range(NCORES)))
    LAST_EXEC_NS = res.exec_time_ns
    global LAST_RES
    LAST_RES = res

    # combine: scatter-add weighted expert outputs (float64 accum)
    combined = np.zeros((N, O), np.float64)
    for c in range(NCORES):
        yc = np.asarray(res.results[c]["out"]).astype(np.float32)  # [128, 6*SC]
        for si, (ri, C, tstart) in enumerate(slots):
            e = assign[c][ri]
            ids = tok_of[e][tstart:tstart + C]
            if not ids:
                continue
            wv = np.asarray(w_of[e][tstart:tstart + C], np.float64)
            blk = yc[:, int(ooffs[si]): int(ooffs[si]) + 6 * C]  # [128, 6C]
            y = blk.reshape(128, 6, C).transpose(1, 0, 2).reshape(O, C)
            y = y[:, :len(ids)].astype(np.float64)
            np.add.at(combined, ids, (y * wv[None, :]).T)

    combined = combined.astype(np.float32)
    mu = combined.mean(-1, keepdims=True)
    var = combined.var(-1, keepdims=True)
    outn = (combined - mu) / np.sqrt(var + 1e-5)
    outn = outn * np.asarray(ln_w, np.float32) + np.asarray(ln_b, np.float32)
    return outn.reshape(B, S, O).astype(np.float32)


# revision 26
# speedup vs baseline: 1.0641x; 1.0641x over previous
"""MoE decoder kernel for Trainium2 (8 NeuronCores, expert-parallel).

Strategy
--------
Host (numpy): gate (sigmoid + top-8 + weight normalization), token->expert
dispatch, weight repacking in PE-friendly layout, final scatter-add
combine + LayerNorm.

Device (Bass/Tile, SPMD over 8 cores): 8 experts per core.  For each
expert the 4-layer MLP runs with *feature-major* activations
(act^T: [feat, tokens]) so that every matmul uses the natural-layout
weight tile [K=128, M=128] as the stationary operand and the activation
tile [K=128, T] as the moving operand -- no transposes anywhere.
Weights stream HBM->SBUF exactly once per expert as large contiguous
"mega chunk" DMAs, double-buffered through ring pools.

W2 (the 2048x2048 layer) and the top 5/8 of W1's contraction dim are
shipped as fp8-e3m4 (4 mantissa bits) with global power-of-2 scales --
W2's scale is undone inside the L2 gelu's `scale` operand, W1's by
pre-scaling the matching x rows on the host (exact in bf16).  The PE
accepts the mixed fp8-stationary x bf16-moving matmul natively.
Outputs store as bf16.  HBM traffic per core: 112MB -> ~70MB, max rel
err ~1.8e-2 (gate: 2e-2; the host sim of this exact quantization has
matched the device result to 4+ digits on every run).

Output stores issue from the GpSimd SWDGE so the sync-ring load stream
is never blocked behind a compute-dependent store.  The first slot's
x/w1 loads are split into k-chunks to shorten the startup ramp, and the
last (smallest) slot is split in half so the final L2/L3/L4 chain
overlaps with remaining compute, shortening the tail.

Per (token,expert) pair only selected pairs are computed (capacity =
per-slot max over cores, rounded to 8), so compute ~= the sparse top-8
workload.
"""

import math

import numpy as np
import ml_dtypes

# problem constants (hardcoded; kernel.py must be self-contained)
B, S, D = 2, 512, 1024
H, BN, O = 2048, 256, 768
E, TOPK = 64, 8
N = B * S
NCORES = 8
EPC = E // NCORES  # experts per core

BF16 = ml_dtypes.bfloat16
E3M4 = ml_dtypes.float8_e3m4

LAST_EXEC_NS = None  # test harness reads this after a traced run
LAST_RES = None  # full BassKernelResults for offline trace analysis


# ---------------------------------------------------------------------------
# host-side routing
# ---------------------------------------------------------------------------

def _route(x, gate_w, gate_bias):
    """Replicates the reference gate in float64: returns top_idx [N,8],
    combine weights wc [N,8] (float32)."""
    xf = x.reshape(N, D).astype(np.float64)
    logits = xf @ gate_w.astype(np.float64).T
    scores = 1.0 / (1.0 + np.exp(-logits))
    choice = scores + gate_bias.astype(np.float64)[None, :]
    # top-8, descending, stable (matches jax.lax.top_k tie behavior)
    top_idx = np.argsort(-choice, axis=1, kind="stable")[:, :TOPK]
    top_scores = np.take_along_axis(choice, top_idx, axis=1)
    wc = top_scores / (top_scores.sum(-1, keepdims=True) + 1e-6)
    return top_idx.astype(np.int64), wc.astype(np.float32)


def _assign_experts(counts):
    """Greedy balance: experts -> cores (EPC slots each), sorted desc within
    a core.  Returns assign[core][slot] = expert id."""
    order = np.argsort(-counts, kind="stable")
    loads = [0] * NCORES
    nslot = [0] * NCORES
    assign = [[] for _ in range(NCORES)]
    for e in order:
        # least-loaded core with a free slot
        c = min(
            (c for c in range(NCORES) if nslot[c] < EPC),
            key=lambda c: (loads[c], c),
        )
        assign[c].append(int(e))
        loads[c] += int(counts[e])
        nslot[c] += 1
    return assign  # each list already desc by count (greedy order)


def _make_slots(caps):
    """Slot schedule: (expert_row ri, cap, token_start).  Experts 0-6 are one
    slot each; the last (smallest) expert is split in two so its final
    L2/L3/L4 chain overlaps the first half's compute."""
    # NOTE: splitting the last slot in half was tried and measured SLOWER
    # (+8us): C=64 matmuls can't hide LDWEIGHTS.  Keep one slot per expert.
    return [(r, int(caps[r]), 0) for r in range(EPC)]


# ---------------------------------------------------------------------------
# device program
# ---------------------------------------------------------------------------

def _build_program(slots, inv_s2):
    import concourse.bass as bass
    import concourse.tile as tile
    from concourse import mybir

    DT = mybir.dt.bfloat16
    F8 = mybir.dt.float8e3
    F32 = mybir.dt.float32
    SC = sum(C for _, C, _ in slots)
    xoffs = np.concatenate([[0], np.cumsum([8 * C for _, C, _ in slots])])
    ooffs = np.concatenate([[0], np.cumsum([6 * C for _, C, _ in slots])])

    nc = bass.Bass(trn_type="TRN2")
    w1s = nc.dram_tensor("w1s", [EPC, 2, 128, 3072], DT, kind="ExternalInput")
    w1f = nc.dram_tensor("w1f", [EPC, 2, 128, 5120], F8, kind="ExternalInput")
    w2s = nc.dram_tensor("w2s", [EPC, 4, 128, 8192], F8, kind="ExternalInput")
    w3s = nc.dram_tensor("w3s", [EPC, 4, 128, 1024], DT, kind="ExternalInput")
    w4s = nc.dram_tensor("w4s", [EPC, 128, 1536], DT, kind="ExternalInput")
    xt = nc.dram_tensor("xt", [128, 8 * SC], DT, kind="ExternalInput")
    bias = nc.dram_tensor("bias", [128, EPC * 40], F32, kind="ExternalInput")
    out = nc.dram_tensor("out", [128, 6 * SC], DT, kind="ExternalOutput")

    GELU = mybir.ActivationFunctionType.Gelu

    with tile.TileContext(nc) as tc:
        with (
            tc.tile_pool(name="wt", bufs=10) as wpool,
            tc.tile_pool(name="wt8", bufs=8) as w8pool,
            tc.tile_pool(name="w1f8", bufs=4) as w1fpool,
            tc.tile_pool(name="xtp", bufs=3) as xpool,
            tc.tile_pool(name="h1p", bufs=32) as h1pool,
            tc.tile_pool(name="h2p", bufs=32) as h2pool,
            tc.tile_pool(name="h3p", bufs=4) as h3pool,
            tc.tile_pool(name="outp", bufs=3) as opool,
            tc.tile_pool(name="ps", bufs=8, space="PSUM") as pspool,
            tc.tile_pool(name="one", bufs=1) as single,
        ):
            bias_sb = single.tile([128, EPC * 40], F32)
            # bias rides the GpSimd ring so the sync ring starts on x/w1
            # immediately
            nc.gpsimd.dma_start(out=bias_sb, in_=bias[:, :])
            # Observer ops: ACT and DVE each touch the bias tile once so the
            # bias-DMA tick is already observed by those engines -- keeps every
            # later activation/tensor_scalar at <=1 sync wait (the legacy
            # walrus codegen rejects instructions with 2+ waits).
            obs_a = single.tile([128, 1], F32)
            nc.scalar.copy(out=obs_a, in_=bias_sb[:, 0:1])
            obs_v = single.tile([128, 1], F32)
            nc.vector.tensor_copy(out=obs_v, in_=bias_sb[:, 0:1])

            psn = [0]

            def psum_alloc(n, C):
                outs = []
                for _ in range(n):
                    psn[0] += 1
                    outs.append(
                        pspool.tile([128, C], F32, tag="ps", name=f"ps{psn[0]}")
                    )
                return outs

            for si, (ri, C, _) in enumerate(slots):
                xoff = int(xoffs[si])
                ooff = int(ooffs[si])
                bcol = ri * 40

                # gathered tokens, transposed: 8 k-tiles of [128, C]
                xtile = xpool.tile([128, 8 * C], DT, tag="xt")
                if si == 0:
                    # startup ramp: land k-tile 0 first so the first matmul
                    # can begin after ~0.3MB instead of ~1.4MB
                    nc.sync.dma_start(out=xtile[:, :C], in_=xt[:, xoff: xoff + C])
                    nc.sync.dma_start(
                        out=xtile[:, C:], in_=xt[:, xoff + C: xoff + 8 * C]
                    )
                else:
                    nc.sync.dma_start(out=xtile, in_=xt[:, xoff: xoff + 8 * C])
                xts = [xtile[:, k * C:(k + 1) * C] for k in range(8)]

                # ---- L1: h1^T[H, C] = gelu(W1^T x + b1), K=D (8 tiles);
                # k0-2 bf16, k3-7 e3m4 (x rows carry the exact 1/s1) ----
                h1 = []
                for g in range(2):  # m-groups of 8 feature tiles
                    psums = psum_alloc(8, C)
                    lo = wpool.tile([128, 3072], DT, tag="wt")
                    if si == 0 and g == 0:
                        for c in range(3):
                            nc.sync.dma_start(
                                out=lo[:, c * 1024:(c + 1) * 1024],
                                in_=w1s[ri, g, :, c * 1024:(c + 1) * 1024],
                            )
                    else:
                        nc.sync.dma_start(out=lo, in_=w1s[ri, g])
                    for c in range(3):
                        k = c
                        for m in range(8):
                            nc.tensor.matmul(
                                psums[m],
                                lo[:, c * 1024 + m * 128: c * 1024 + (m + 1) * 128],
                                xts[k],
                                start=(k == 0),
                                stop=False,
                            )
                    hi = w1fpool.tile([128, 5120], F8, tag="w1f8")
                    nc.sync.dma_start(out=hi, in_=w1f[ri, g])
                    for c in range(5):
                        k = 3 + c
                        for m in range(8):
                            nc.tensor.matmul(
                                psums[m],
                                hi[:, c * 1024 + m * 128: c * 1024 + (m + 1) * 128],
                                xts[k],
                                start=False,
                                stop=(k == 7),
                            )
                    for m in range(8):
                        hh = h1pool.tile([128, C], DT, tag="h1")
                        nc.scalar.activation(
                            out=hh, in_=psums[m], func=GELU,
                            bias=bias_sb[:, bcol + g * 8 + m: bcol + g * 8 + m + 1],
                        )
                        h1.append(hh)

                # ---- L2: h2^T[H, C] = gelu(W2^T h1 + b2), K=H (16 tiles),
                # W2 in fp8-e3m4 scaled by s2 (undone via activation scale) --
                h2 = []
                for g in range(2):
                    psums = psum_alloc(8, C)
                    for mega in range(2):  # 2 megas x 8 k-chunks
                        wt = w8pool.tile([128, 8192], F8, tag="wt8")
                        nc.sync.dma_start(out=wt, in_=w2s[ri, g * 2 + mega])
                        for c in range(8):
                            k = mega * 8 + c
                            for m in range(8):
                                nc.tensor.matmul(
                                    psums[m],
                                    wt[:, c * 1024 + m * 128: c * 1024 + (m + 1) * 128],
                                    h1[k],
                                    start=(k == 0),
                                    stop=(k == 15),
                                )
                    for m in range(8):
                        hh = h2pool.tile([128, C], DT, tag="h2")
                        nc.scalar.activation(
                            out=hh, in_=psums[m], func=GELU, scale=float(inv_s2),
                            bias=bias_sb[:, bcol + 16 + g * 8 + m: bcol + 16 + g * 8 + m + 1],
                        )
                        h2.append(hh)

                # ---- L3: h3^T[BN, C] = W3^T h2 + b3, K=H (16 tiles) ----
                psums3 = psum_alloc(2, C)
                for mega in range(4):
                    wt = wpool.tile([128, 1024], DT, tag="wt")
                    nc.sync.dma_start(out=wt, in_=w3s[ri, mega])
                    for c in range(4):
                        k = mega * 4 + c
                        for m in range(2):
                            nc.tensor.matmul(
                                psums3[m],
                                wt[:, c * 256 + m * 128: c * 256 + (m + 1) * 128],
                                h2[k],
                                start=(k == 0),
                                stop=(k == 15),
                            )
                h3 = []
                for m in range(2):
                    hh = h3pool.tile([128, C], DT, tag="h3")
                    nc.vector.tensor_scalar_add(
                        hh, psums3[m], bias_sb[:, bcol + 32 + m: bcol + 32 + m + 1]
                    )
                    h3.append(hh)

                # ---- L4: out^T[O, C] = W4^T h3 + b4, K=BN (2 tiles) ----
                psums4 = psum_alloc(6, C)
                wt = wpool.tile([128, 1536], DT, tag="wt")
                nc.sync.dma_start(out=wt, in_=w4s[ri])
                for c in range(2):
                    for m in range(6):
                        nc.tensor.matmul(
                            psums4[m],
                            wt[:, c * 768 + m * 128: c * 768 + (m + 1) * 128],
                            h3[c],
                            start=(c == 0),
                            stop=(c == 1),
                        )
                ot = opool.tile([128, 6 * C], DT, tag="out")
                for m in range(6):
                    nc.vector.tensor_scalar_add(
                        ot[:, m * C:(m + 1) * C], psums4[m],
                        bias_sb[:, bcol + 34 + m: bcol + 34 + m + 1],
                    )
                # store via GpSimd (SWDGE): a store's sequencer-side wait on
                # the DVE adds must not stall the sync-engine FIFO, which
                # carries every weight load for the NEXT experts.
                nc.gpsimd.dma_start(
                    out=out[:, ooff: ooff + 6 * C], in_=ot
                )

    _legalize_waits(nc, mybir)
    return nc


def _legalize_waits(nc, mybir):
    """The legacy walrus codegen (bass2jax path) rejects instructions carrying
    more than one sync wait.  Split every multi-wait instruction: hoist all
    but the last wait onto same-engine InstNoOp carriers inserted just before
    it (engine program order preserves the gating semantics)."""
    n = 0
    for bb in nc.main_func.blocks:
        insts = bb.instructions
        i = 0
        while i < len(insts):
            ins = insts[i]
            si = ins.sync_info
            if si is not None and si.on_wait and len(si.on_wait) > 1:
                extra = list(si.on_wait[:-1])
                keep = [si.on_wait[-1]]
                for w in extra:
                    noop = mybir.InstNoOp(
                        name=f"NOPW-{n}", engine=ins.engine, ins=[], outs=[],
                        sync_info=mybir.SyncInfo(on_wait=[w], on_update=[]),
                    )
                    n += 1
                    insts.insert(i, noop)
                    i += 1
                ins.sync_info = mybir.SyncInfo(
                    on_wait=keep, on_update=list(si.on_update or [])
                )
            i += 1


# ---------------------------------------------------------------------------
# host-side packing
# ---------------------------------------------------------------------------

def _pack_core(w1, b1, w2, b2, w3, b3, w4, b4, experts, s1, s2):
    """Pack one core's 8 experts into the DRAM layouts the program expects."""
    idx = np.asarray(experts)
    # W1 [e,1024,2048]: chunks (g,k) of [128,1024]; k0-2 -> bf16 mega,
    # k3-7 -> e3m4 mega scaled by s1 (undone via the 1/s1-scaled x rows)
    a = w1[idx].reshape(EPC, 8, 128, 2, 1024)
    a = a.transpose(0, 3, 1, 2, 4)  # [EPC, g, k, 128, 1024]
    lo = a[:, :, :3].transpose(0, 1, 3, 2, 4).reshape(EPC, 2, 128, 3072)
    hi = a[:, :, 3:].transpose(0, 1, 3, 2, 4).reshape(EPC, 2, 128, 5120)
    w1p = np.ascontiguousarray(lo).astype(BF16)
    w1fp = np.clip(np.ascontiguousarray(hi) * s1, -15.5, 15.5).astype(E3M4)

    # W2 [e,2048,2048] -> fp8-e3m4 [e,4,128,8192]: 8 k-chunks of [128,1024]
    # per mega, scaled by s2
    a = w2[idx].reshape(EPC, 16, 128, 2, 1024)
    a = a.transpose(0, 3, 1, 2, 4).reshape(EPC, 32, 128, 1024)
    a = a.reshape(EPC, 4, 8, 128, 1024).transpose(0, 1, 3, 2, 4)
    w2p = np.clip(
        np.ascontiguousarray(a).reshape(EPC, 4, 128, 8192) * s2, -15.5, 15.5
    ).astype(E3M4)

    a = w3[idx].reshape(EPC, 16, 128, 256)  # k-chunks of [128,256]
    w3p = np.ascontiguousarray(
        a.reshape(EPC, 4, 4, 128, 256).transpose(0, 1, 3, 2, 4)
    ).reshape(EPC, 4, 128, 1024).astype(BF16)

    a = w4[idx].reshape(EPC, 2, 128, 768)
    w4p = np.ascontiguousarray(a.transpose(0, 2, 1, 3)).reshape(
        EPC, 128, 1536
    ).astype(BF16)

    # biases: per expert 40 cols of [128]: L1 m0-15 | L2 m0-15 | L3 m0-1 | L4 m0-5
    bb = np.concatenate(
        [
            b1[idx].reshape(EPC, 16, 128),
            b2[idx].reshape(EPC, 16, 128),
            b3[idx].reshape(EPC, 2, 128),
            b4[idx].reshape(EPC, 6, 128),
        ],
        axis=1,
    )  # [EPC, 40, 128]
    biasp = np.ascontiguousarray(
        bb.reshape(EPC * 40, 128).T
    ).astype(np.float32)  # [128, EPC*40]
    return w1p, w1fp, w2p, w3p, w4p, biasp


def kernel(x, gate_w, gate_bias, w1, b1, w2, b2, w3, b3, w4, b4, ln_w, ln_b):
    global LAST_EXEC_NS
    x = np.asarray(x, np.float32)
    xf = x.reshape(N, D)

    top_idx, wc = _route(x, np.asarray(gate_w, np.float32),
                         np.asarray(gate_bias, np.float32))

    # token lists per expert
    counts = np.bincount(top_idx.ravel(), minlength=E)
    tok_of = [[] for _ in range(E)]
    w_of = [[] for _ in range(E)]
    flat_tok = np.repeat(np.arange(N), TOPK)
    flat_exp = top_idx.ravel()
    flat_w = wc.ravel()
    order = np.argsort(flat_exp, kind="stable")
    for t, e, w in zip(flat_tok[order], flat_exp[order], flat_w[order]):
        tok_of[e].append(int(t))
        w_of[e].append(float(w))

    assign = _assign_experts(counts)

    # per-slot capacities (shared across cores; slots sorted desc by count)
    caps = np.zeros(EPC, int)
    for c in range(NCORES):
        for r, e in enumerate(assign[c]):
            caps[r] = max(caps[r], counts[e])
    caps = ((caps + 7) // 8) * 8
    slots = _make_slots(caps)
    SC = sum(C for _, C, _ in slots)
    xoffs = np.concatenate([[0], np.cumsum([8 * C for _, C, _ in slots])])
    ooffs = np.concatenate([[0], np.cumsum([6 * C for _, C, _ in slots])])

    w1a = np.asarray(w1, np.float32); b1a = np.asarray(b1, np.float32)
    w2a = np.asarray(w2, np.float32); b2a = np.asarray(b2, np.float32)
    w3a = np.asarray(w3, np.float32); b3a = np.asarray(b3, np.float32)
    w4a = np.asarray(w4, np.float32); b4a = np.asarray(b4, np.float32)

    # global power-of-2 scales for the fp8-e3m4 weights (max normal 15.5)
    s1 = 2.0 ** math.floor(math.log2(15.0 / float(np.abs(w1a).max())))
    s2 = 2.0 ** math.floor(math.log2(15.0 / float(np.abs(w2a).max())))

    nc = _build_program(slots, 1.0 / s2)

    xt_bf = xf.T.astype(BF16)  # [D, N]
    # x rows 384-1023 (k-tiles 3-7) carry the exact 1/s1 to undo the fp8
    # part of W1's scale; power-of-2 so the bf16 cast stays lossless
    xt_bf[384:] = (xt_bf[384:].astype(np.float32) * (1.0 / s1)).astype(BF16)
    in_maps = []
    for c in range(NCORES):
        w1p, w1fp, w2p, w3p, w4p, biasp = _pack_core(
            w1a, b1a, w2a, b2a, w3a, b3a, w4a, b4a, assign[c], s1, s2
        )
        # xt layout: per slot, 8 k-tiles [128, C] side by side
        xtc = np.zeros((128, 8 * int(SC)), BF16)
        for si, (ri, C, tstart) in enumerate(slots):
            e = assign[c][ri]
            ids = tok_of[e][tstart:tstart + C]
            if not ids:
                continue
            blk = xt_bf[:, ids]  # [D, n]
            for k in range(8):
                base = int(xoffs[si]) + k * C
                xtc[:, base: base + len(ids)] = blk[k * 128:(k + 1) * 128]
        in_maps.append(
            {"w1s": w1p, "w1f": w1fp, "w2s": w2p, "w3s": w3p, "w4s": w4p,
             "xt": xtc, "bias": biasp}
        )

    from concourse.bass_utils import run_bass_kernel_spmd

    res = run_bass_kernel_spmd(nc, in_maps, core_ids=list(range(NCORES)))
    LAST_EXEC_NS = res.exec_time_ns
    global LAST_RES
    LAST_RES = res

    # combine: scatter-add weighted expert outputs (float64 accum)
    combined = np.zeros((N, O), np.float64)
    for c in range(NCORES):
        yc = np.asarray(res.results[c]["out"]).astype(np.float32)  # [128, 6*SC]
        for si, (ri, C, tstart) in enumerate(slots):
            e = assign[c][ri]
            ids = tok_of[e][tstart:tstart + C]
            if not ids:
                continue
            wv = np.asarray(w_of[e][tstart:tstart + C], np.float64)
            blk = yc[:, int(ooffs[si]): int(ooffs[si]) + 6 * C]  # [128, 6C]
            y = blk.reshape(128, 6, C).transpose(1, 0, 2).reshape(O, C)
            y = y[:, :len(ids)].astype(np.float64)
            np.add.at(combined, ids, (y * wv[None, :]).T)

    combined = combined.astype(np.float32)
    mu = combined.mean(-1, keepdims=True)
    var = combined.var(-1, keepdims=True)
    outn = (combined - mu) / np.sqrt(var + 1e-5)
    outn = outn * np.asarray(ln_w, np.float32) + np.asarray(ln_b, np.float32)
    return outn.reshape(B, S, O).astype(np.float32)


# revision 29
# speedup vs baseline: 1.0706x; 1.0062x over previous
"""MoE decoder kernel for Trainium2 (8 NeuronCores, expert-parallel).

Strategy
--------
Host (numpy): gate (sigmoid + top-8 + weight normalization), token->expert
dispatch, weight repacking in PE-friendly layout, final scatter-add
combine + LayerNorm.

Device (Bass/Tile, SPMD over 8 cores): 8 experts per core.  For each
expert the 4-layer MLP runs with *feature-major* activations
(act^T: [feat, tokens]) so that every matmul uses the natural-layout
weight tile [K=128, M=128] as the stationary operand and the activation
tile [K=128, T] as the moving operand -- no transposes anywhere.
Weights stream HBM->SBUF exactly once per expert as large contiguous
"mega chunk" DMAs, double-buffered through ring pools.

W2 (the 2048x2048 layer) and the top 5/8 of W1's contraction dim are
shipped as fp8-e3m4 (4 mantissa bits) with global power-of-2 scales --
W2's scale is undone inside the L2 gelu's `scale` operand, W1's by
pre-scaling the matching x rows on the host (exact in bf16).  The PE
accepts the mixed fp8-stationary x bf16-moving matmul natively.
Outputs store as bf16.  HBM traffic per core: 112MB -> ~70MB, max rel
err ~1.8e-2 (gate: 2e-2; the host sim of this exact quantization has
matched the device result to 4+ digits on every run).

Output stores issue from the GpSimd SWDGE so the sync-ring load stream
is never blocked behind a compute-dependent store.  The first slot's
x/w1 loads are split into k-chunks to shorten the startup ramp, and the
last (smallest) slot is split in half so the final L2/L3/L4 chain
overlaps with remaining compute, shortening the tail.

Per (token,expert) pair only selected pairs are computed (capacity =
per-slot max over cores, rounded to 8), so compute ~= the sparse top-8
workload.
"""

import math

import numpy as np
import ml_dtypes

# problem constants (hardcoded; kernel.py must be self-contained)
B, S, D = 2, 512, 1024
H, BN, O = 2048, 256, 768
E, TOPK = 64, 8
N = B * S
NCORES = 8
EPC = E // NCORES  # experts per core

BF16 = ml_dtypes.bfloat16
E3M4 = ml_dtypes.float8_e3m4

LAST_EXEC_NS = None  # test harness reads this after a traced run
LAST_RES = None  # full BassKernelResults for offline trace analysis


# ---------------------------------------------------------------------------
# host-side routing
# ---------------------------------------------------------------------------

def _route(x, gate_w, gate_bias):
    """Replicates the reference gate in float64: returns top_idx [N,8],
    combine weights wc [N,8] (float32)."""
    xf = x.reshape(N, D).astype(np.float64)
    logits = xf @ gate_w.astype(np.float64).T
    scores = 1.0 / (1.0 + np.exp(-logits))
    choice = scores + gate_bias.astype(np.float64)[None, :]
    # top-8, descending, stable (matches jax.lax.top_k tie behavior)
    top_idx = np.argsort(-choice, axis=1, kind="stable")[:, :TOPK]
    top_scores = np.take_along_axis(choice, top_idx, axis=1)
    wc = top_scores / (top_scores.sum(-1, keepdims=True) + 1e-6)
    return top_idx.astype(np.int64), wc.astype(np.float32)


def _assign_experts(counts):
    """Greedy balance: experts -> cores (EPC slots each), sorted desc within
    a core.  Returns assign[core][slot] = expert id."""
    order = np.argsort(-counts, kind="stable")
    loads = [0] * NCORES
    nslot = [0] * NCORES
    assign = [[] for _ in range(NCORES)]
    for e in order:
        # least-loaded core with a free slot
        c = min(
            (c for c in range(NCORES) if nslot[c] < EPC),
            key=lambda c: (loads[c], c),
        )
        assign[c].append(int(e))
        loads[c] += int(counts[e])
        nslot[c] += 1
    return assign  # each list already desc by count (greedy order)


def _make_slots(caps):
    """Slot schedule: (expert_row ri, cap, token_start).  Experts 0-6 are one
    slot each; the last (smallest) expert is split in two so its final
    L2/L3/L4 chain overlaps the first half's compute."""
    # NOTE: splitting the last slot in half was tried and measured SLOWER
    # (+8us): C=64 matmuls can't hide LDWEIGHTS.  Keep one slot per expert.
    return [(r, int(caps[r]), 0) for r in range(EPC)]


# ---------------------------------------------------------------------------
# device program
# ---------------------------------------------------------------------------

def _build_program(slots, inv_s2):
    import concourse.bass as bass
    import concourse.tile as tile
    from concourse import mybir

    DT = mybir.dt.bfloat16
    F8 = mybir.dt.float8e3
    F32 = mybir.dt.float32
    SC = sum(C for _, C, _ in slots)
    xoffs = np.concatenate([[0], np.cumsum([8 * C for _, C, _ in slots])])
    ooffs = np.concatenate([[0], np.cumsum([6 * C for _, C, _ in slots])])

    nc = bass.Bass(trn_type="TRN2")
    w1s = nc.dram_tensor("w1s", [EPC, 2, 128, 3072], DT, kind="ExternalInput")
    w1f = nc.dram_tensor("w1f", [EPC, 2, 128, 5120], F8, kind="ExternalInput")
    w2s = nc.dram_tensor("w2s", [EPC, 4, 128, 8192], F8, kind="ExternalInput")
    w3s = nc.dram_tensor("w3s", [EPC, 4, 128, 1024], DT, kind="ExternalInput")
    w4s = nc.dram_tensor("w4s", [EPC, 128, 1536], DT, kind="ExternalInput")
    xt = nc.dram_tensor("xt", [128, 8 * SC], DT, kind="ExternalInput")
    bias = nc.dram_tensor("bias", [128, EPC * 40], F32, kind="ExternalInput")
    out = nc.dram_tensor("out", [128, 6 * SC], DT, kind="ExternalOutput")

    GELU = mybir.ActivationFunctionType.Gelu

    with tile.TileContext(nc) as tc:
        with (
            tc.tile_pool(name="wt", bufs=10) as wpool,
            tc.tile_pool(name="wt8", bufs=8) as w8pool,
            tc.tile_pool(name="w1f8", bufs=4) as w1fpool,
            tc.tile_pool(name="xtp", bufs=3) as xpool,
            tc.tile_pool(name="h1p", bufs=32) as h1pool,
            tc.tile_pool(name="h2p", bufs=32) as h2pool,
            tc.tile_pool(name="h3p", bufs=4) as h3pool,
            tc.tile_pool(name="outp", bufs=3) as opool,
            tc.tile_pool(name="ps", bufs=8, space="PSUM") as pspool,
            tc.tile_pool(name="one", bufs=1) as single,
        ):
            bias_sb = single.tile([128, EPC * 40], F32)
            # bias rides the GpSimd ring so the sync ring starts on x/w1
            # immediately
            nc.gpsimd.dma_start(out=bias_sb, in_=bias[:, :])
            # Observer ops: ACT and DVE each touch the bias tile once so the
            # bias-DMA tick is already observed by those engines -- keeps every
            # later activation/tensor_scalar at <=1 sync wait (the legacy
            # walrus codegen rejects instructions with 2+ waits).
            obs_a = single.tile([128, 1], F32)
            nc.scalar.copy(out=obs_a, in_=bias_sb[:, 0:1])
            obs_v = single.tile([128, 1], F32)
            nc.vector.tensor_copy(out=obs_v, in_=bias_sb[:, 0:1])

            psn = [0]

            def psum_alloc(n, C):
                outs = []
                for _ in range(n):
                    psn[0] += 1
                    outs.append(
                        pspool.tile([128, C], F32, tag="ps", name=f"ps{psn[0]}")
                    )
                return outs

            for si, (ri, C, _) in enumerate(slots):
                xoff = int(xoffs[si])
                ooff = int(ooffs[si])
                bcol = ri * 40

                # gathered tokens, transposed: 8 k-tiles of [128, C]
                xtile = xpool.tile([128, 8 * C], DT, tag="xt")
                if si == 0:
                    # startup ramp: land k-tile 0 first so the first matmul
                    # can begin after ~0.3MB instead of ~1.4MB
                    nc.sync.dma_start(out=xtile[:, :C], in_=xt[:, xoff: xoff + C])
                    nc.sync.dma_start(
                        out=xtile[:, C:], in_=xt[:, xoff + C: xoff + 8 * C]
                    )
                else:
                    nc.sync.dma_start(out=xtile, in_=xt[:, xoff: xoff + 8 * C])
                xts = [xtile[:, k * C:(k + 1) * C] for k in range(8)]

                # ---- L1: h1^T[H, C] = gelu(W1^T x + b1), K=D (8 tiles);
                # k0-2 bf16, k3-7 e3m4 (x rows carry the exact 1/s1) ----
                h1 = []
                for g in range(2):  # m-groups of 8 feature tiles
                    psums = psum_alloc(8, C)
                    lo = wpool.tile([128, 3072], DT, tag="wt")
                    if si == 0 and g == 0:
                        for c in range(3):
                            nc.sync.dma_start(
                                out=lo[:, c * 1024:(c + 1) * 1024],
                                in_=w1s[ri, g, :, c * 1024:(c + 1) * 1024],
                            )
                    else:
                        nc.sync.dma_start(out=lo, in_=w1s[ri, g])
                    for c in range(3):
                        k = c
                        for m in range(8):
                            nc.tensor.matmul(
                                psums[m],
                                lo[:, c * 1024 + m * 128: c * 1024 + (m + 1) * 128],
                                xts[k],
                                start=(k == 0),
                                stop=False,
                            )
                    hi = w1fpool.tile([128, 5120], F8, tag="w1f8")
                    nc.sync.dma_start(out=hi, in_=w1f[ri, g])
                    for c in range(5):
                        k = 3 + c
                        for m in range(8):
                            nc.tensor.matmul(
                                psums[m],
                                hi[:, c * 1024 + m * 128: c * 1024 + (m + 1) * 128],
                                xts[k],
                                start=False,
                                stop=(k == 7),
                            )
                    for m in range(8):
                        hh = h1pool.tile([128, C], DT, tag="h1")
                        nc.scalar.activation(
                            out=hh, in_=psums[m], func=GELU,
                            bias=bias_sb[:, bcol + g * 8 + m: bcol + g * 8 + m + 1],
                        )
                        h1.append(hh)

                # ---- L2: h2^T[H, C] = gelu(W2^T h1 + b2), K=H (16 tiles),
                # W2 in fp8-e3m4 scaled by s2 (undone via activation scale) --
                h2 = []
                for g in range(2):
                    psums = psum_alloc(8, C)
                    for mega in range(2):  # 2 megas x 8 k-chunks
                        wt = w8pool.tile([128, 8192], F8, tag="wt8")
                        nc.sync.dma_start(out=wt, in_=w2s[ri, g * 2 + mega])
                        for c in range(8):
                            k = mega * 8 + c
                            for m in range(8):
                                nc.tensor.matmul(
                                    psums[m],
                                    wt[:, c * 1024 + m * 128: c * 1024 + (m + 1) * 128],
                                    h1[k],
                                    start=(k == 0),
                                    stop=(k == 15),
                                )
                    for m in range(8):
                        hh = h2pool.tile([128, C], DT, tag="h2")
                        nc.scalar.activation(
                            out=hh, in_=psums[m], func=GELU, scale=float(inv_s2),
                            bias=bias_sb[:, bcol + 16 + g * 8 + m: bcol + 16 + g * 8 + m + 1],
                        )
                        h2.append(hh)

                # ---- L3: h3^T[BN, C] = W3^T h2 + b3, K=H (16 tiles) ----
                psums3 = psum_alloc(2, C)
                for mega in range(4):
                    wt = wpool.tile([128, 1024], DT, tag="wt")
                    nc.sync.dma_start(out=wt, in_=w3s[ri, mega])
                    for c in range(4):
                        k = mega * 4 + c
                        for m in range(2):
                            nc.tensor.matmul(
                                psums3[m],
                                wt[:, c * 256 + m * 128: c * 256 + (m + 1) * 128],
                                h2[k],
                                start=(k == 0),
                                stop=(k == 15),
                            )
                h3 = []
                for m in range(2):
                    hh = h3pool.tile([128, C], DT, tag="h3")
                    nc.vector.tensor_scalar_add(
                        hh, psums3[m], bias_sb[:, bcol + 32 + m: bcol + 32 + m + 1]
                    )
                    h3.append(hh)

                # ---- L4: out^T[O, C] = W4^T h3 + b4, K=BN (2 tiles) ----
                psums4 = psum_alloc(6, C)
                wt = wpool.tile([128, 1536], DT, tag="wt")
                nc.sync.dma_start(out=wt, in_=w4s[ri])
                for c in range(2):
                    for m in range(6):
                        nc.tensor.matmul(
                            psums4[m],
                            wt[:, c * 768 + m * 128: c * 768 + (m + 1) * 128],
                            h3[c],
                            start=(c == 0),
                            stop=(c == 1),
                        )
                ot = opool.tile([128, 6 * C], DT, tag="out")
                for m in range(6):
                    nc.vector.tensor_scalar_add(
                        ot[:, m * C:(m + 1) * C], psums4[m],
                        bias_sb[:, bcol + 34 + m: bcol + 34 + m + 1],
                    )
                # store via GpSimd (SWDGE): a store's sequencer-side wait on
                # the DVE adds must not stall the sync-engine FIFO, which
                # carries every weight load for the NEXT experts.
                nc.gpsimd.dma_start(
                    out=out[:, ooff: ooff + 6 * C], in_=ot
                )

    _legalize_waits(nc, mybir)
    return nc


def _legalize_waits(nc, mybir):
    """The legacy walrus codegen (bass2jax path) rejects instructions carrying
    more than one sync wait.  Split every multi-wait instruction: hoist all
    but the last wait onto same-engine InstNoOp carriers inserted just before
    it (engine program order preserves the gating semantics)."""
    n = 0
    for bb in nc.main_func.blocks:
        insts = bb.instructions
        i = 0
        while i < len(insts):
            ins = insts[i]
            si = ins.sync_info
            if si is not None and si.on_wait and len(si.on_wait) > 1:
                extra = list(si.on_wait[:-1])
                keep = [si.on_wait[-1]]
                for w in extra:
                    noop = mybir.InstNoOp(
                        name=f"NOPW-{n}", engine=ins.engine, ins=[], outs=[],
                        sync_info=mybir.SyncInfo(on_wait=[w], on_update=[]),
                    )
                    n += 1
                    insts.insert(i, noop)
                    i += 1
                ins.sync_info = mybir.SyncInfo(
                    on_wait=keep, on_update=list(si.on_update or [])
                )
            i += 1


# ---------------------------------------------------------------------------
# host-side packing
# ---------------------------------------------------------------------------

def _pack_core(w1, b1, w2, b2, w3, b3, w4, b4, experts, s1, s2):
    """Pack one core's 8 experts into the DRAM layouts the program expects."""
    idx = np.asarray(experts)
    # W1 [e,1024,2048]: chunks (g,k) of [128,1024]; k0-2 -> bf16 mega,
    # k3-7 -> e3m4 mega scaled by s1 (undone via the 1/s1-scaled x rows)
    a = w1[idx].reshape(EPC, 8, 128, 2, 1024)
    a = a.transpose(0, 3, 1, 2, 4)  # [EPC, g, k, 128, 1024]
    lo = a[:, :, :3].transpose(0, 1, 3, 2, 4).reshape(EPC, 2, 128, 3072)
    hi = a[:, :, 3:].transpose(0, 1, 3, 2, 4).reshape(EPC, 2, 128, 5120)
    w1p = np.ascontiguousarray(lo).astype(BF16)
    w1fp = np.clip(np.ascontiguousarray(hi) * s1, -15.5, 15.5).astype(E3M4)

    # W2 [e,2048,2048] -> fp8-e3m4 [e,4,128,8192]: 8 k-chunks of [128,1024]
    # per mega, scaled by s2
    a = w2[idx].reshape(EPC, 16, 128, 2, 1024)
    a = a.transpose(0, 3, 1, 2, 4).reshape(EPC, 32, 128, 1024)
    a = a.reshape(EPC, 4, 8, 128, 1024).transpose(0, 1, 3, 2, 4)
    w2p = np.clip(
        np.ascontiguousarray(a).reshape(EPC, 4, 128, 8192) * s2, -15.5, 15.5
    ).astype(E3M4)

    a = w3[idx].reshape(EPC, 16, 128, 256)  # k-chunks of [128,256]
    w3p = np.ascontiguousarray(
        a.reshape(EPC, 4, 4, 128, 256).transpose(0, 1, 3, 2, 4)
    ).reshape(EPC, 4, 128, 1024).astype(BF16)

    a = w4[idx].reshape(EPC, 2, 128, 768)
    w4p = np.ascontiguousarray(a.transpose(0, 2, 1, 3)).reshape(
        EPC, 128, 1536
    ).astype(BF16)

    # biases: per expert 40 cols of [128]: L1 m0-15 | L2 m0-15 | L3 m0-1 | L4 m0-5
    bb = np.concatenate(
        [
            b1[idx].reshape(EPC, 16, 128),
            b2[idx].reshape(EPC, 16, 128),
            b3[idx].reshape(EPC, 2, 128),
            b4[idx].reshape(EPC, 6, 128),
        ],
        axis=1,
    )  # [EPC, 40, 128]
    biasp = np.ascontiguousarray(
        bb.reshape(EPC * 40, 128).T
    ).astype(np.float32)  # [128, EPC*40]
    return w1p, w1fp, w2p, w3p, w4p, biasp


def kernel(x, gate_w, gate_bias, w1, b1, w2, b2, w3, b3, w4, b4, ln_w, ln_b):
    global LAST_EXEC_NS
    x = np.asarray(x, np.float32)
    xf = x.reshape(N, D)

    top_idx, wc = _route(x, np.asarray(gate_w, np.float32),
                         np.asarray(gate_bias, np.float32))

    # token lists per expert
    counts = np.bincount(top_idx.ravel(), minlength=E)
    tok_of = [[] for _ in range(E)]
    w_of = [[] for _ in range(E)]
    flat_tok = np.repeat(np.arange(N), TOPK)
    flat_exp = top_idx.ravel()
    flat_w = wc.ravel()
    order = np.argsort(flat_exp, kind="stable")
    for t, e, w in zip(flat_tok[order], flat_exp[order], flat_w[order]):
        tok_of[e].append(int(t))
        w_of[e].append(float(w))

    assign = _assign_experts(counts)

    # per-slot capacities (shared across cores; slots sorted desc by count)
    caps = np.zeros(EPC, int)
    for c in range(NCORES):
        for r, e in enumerate(assign[c]):
            caps[r] = max(caps[r], counts[e])
    caps = ((caps + 7) // 8) * 8
    slots = _make_slots(caps)
    SC = sum(C for _, C, _ in slots)
    xoffs = np.concatenate([[0], np.cumsum([8 * C for _, C, _ in slots])])
    ooffs = np.concatenate([[0], np.cumsum([6 * C for _, C, _ in slots])])

    w1a = np.asarray(w1, np.float32); b1a = np.asarray(b1, np.float32)
    w2a = np.asarray(w2, np.float32); b2a = np.asarray(b2, np.float32)
    w3a = np.asarray(w3, np.float32); b3a = np.asarray(b3, np.float32)
    w4a = np.asarray(w4, np.float32); b4a = np.asarray(b4, np.float32)

    # global power-of-2 scales for the fp8-e3m4 weights (max normal 15.5)
    s1 = 2.0 ** math.floor(math.log2(15.0 / float(np.abs(w1a).max())))
    s2 = 2.0 ** math.floor(math.log2(15.0 / float(np.abs(w2a).max())))

    nc = _build_program(slots, 1.0 / s2)

    xt_bf = xf.T.astype(BF16)  # [D, N]
    # x rows 384-1023 (k-tiles 3-7) carry the exact 1/s1 to undo the fp8
    # part of W1's scale; power-of-2 so the bf16 cast stays lossless
    xt_bf[384:] = (xt_bf[384:].astype(np.float32) * (1.0 / s1)).astype(BF16)
    in_maps = []
    for c in range(NCORES):
        w1p, w1fp, w2p, w3p, w4p, biasp = _pack_core(
            w1a, b1a, w2a, b2a, w3a, b3a, w4a, b4a, assign[c], s1, s2
        )
        # xt layout: per slot, 8 k-tiles [128, C] side by side
        xtc = np.zeros((128, 8 * int(SC)), BF16)
        for si, (ri, C, tstart) in enumerate(slots):
            e = assign[c][ri]
            ids = tok_of[e][tstart:tstart + C]
            if not ids:
                continue
            blk = xt_bf[:, ids]  # [D, n]
            for k in range(8):
                base = int(xoffs[si]) + k * C
                xtc[:, base: base + len(ids)] = blk[k * 128:(k + 1) * 128]
        in_maps.append(
            {"w1s": w1p, "w1f": w1fp, "w2s": w2p, "w3s": w3p, "w4s": w4p,
             "xt": xtc, "bias": biasp}
        )

    from concourse.bass_utils import run_bass_kernel_spmd

    res = run_bass_kernel_spmd(nc, in_maps, core_ids=list(range(NCORES)))
    LAST_EXEC_NS = res.exec_time_ns
    global LAST_RES
    LAST_RES = res

    # combine: scatter-add weighted expert outputs (float64 accum)
    combined = np.zeros((N, O), np.float64)
    for c in range(NCORES):
        yc = np.asarray(res.results[c]["out"]).astype(np.float32)  # [128, 6*SC]
        for si, (ri, C, tstart) in enumerate(slots):
            e = assign[c][ri]
            ids = tok_of[e][tstart:tstart + C]
            if not ids:
                continue
            wv = np.asarray(w_of[e][tstart:tstart + C], np.float64)
            blk = yc[:, int(ooffs[si]): int(ooffs[si]) + 6 * C]  # [128, 6C]
            y = blk.reshape(128, 6, C).transpose(1, 0, 2).reshape(O, C)
            y = y[:, :len(ids)].astype(np.float64)
            np.add.at(combined, ids, (y * wv[None, :]).T)

    combined = combined.astype(np.float32)
    mu = combined.mean(-1, keepdims=True)
    var = combined.var(-1, keepdims=True)
    outn = (combined - mu) / np.sqrt(var + 1e-5)
    outn = outn * np.asarray(ln_w, np.float32) + np.asarray(ln_b, np.float32)
    return outn.reshape(B, S, O).astype(np.float32)
